# revision 2
# baseline (speedup 1.0000x reference)
"""Causal self-attention (SEQ=8192, D=1024) on 8 TRN2 NeuronCores.

Strategy (SPMD, one static graph on all 8 cores), v7 "raw-keys" design:
  - Sequence parallel over queries with stride-8 row interleaving:
    core i owns query rows {8j+i : j in [0,1024)}. This balances causal
    work exactly while keeping the instruction graph identical across
    cores (per-core differences are pure data: X^T slices + masks).
  - The combined score weight is applied on the QUERY side:
    scores = (x_q M) x_k^T with M = Wq^T Wk, so the key-side ST operand
    is the RAW input X^T -- which every core receives as a (replicated)
    input. No K projection and no K all-gathers at all; ST reads key
    chunks straight from DRAM with no collective in the way.
  - Core i computes V for its contiguous key shard [1024*i, 1024*(i+1))
    only; V is shared via THREE chunked AllGathers (V-h0 in two 256-key
    pieces, V-h1). The CC engine takes ~70us to start its first
    collective regardless of trigger time, and the chain finishes by
    ~200us -- well before PV needs each piece.
  - ALL key-chunk tiles ([128, 8, 256] from X^T) stream through ONE
    pool (kp, 14 bufs) and ALL gathered-V tiles ([128, 2, 1024])
    through ONE pool (vp, 16 bufs): uniform tile sizes mean h1 data
    reuses h0 slots with natural WAR ordering, SBUF stays ~200 KB in
    every phase, and no read is throttled by pool-lifetime conflicts.
    kp opens before io (virgin SBUF: kt0 reads issue at t~10us); vp
    opens right after io closes and inherits its region (reads fire
    the moment the projections drain).
  - Projection order: V-h0 (dh=0 groups first: only wv's first half
    gates the start), V-h1, then Qt = x_q M evicted to SBUF.
  - Attention runs in S^T layout ([keys x queries]): S^T = Xk^T.T @ Qt^T,
    so softmax(P)^T is directly the lhsT for P@V -- no transposes.
    Per key-half: ST for all shards first (H1_ORDER interleaves tiny
    diagonal groups between big ones so the exp activations keep
    pace), then PV in J-major order with one PSUM accumulation group
    per (J, kt-piece) spanning all shards.  exp on ScalarE (scale
    fused), no max-subtraction (scores are N(0,1)-scaled).
  - Denominators: a [128, 8] PSUM tile accumulates one ones-COLUMN
    matmul per PV chunk (lhsT = the P tile already stationary for the
    PV matmuls, rhs = ones[128,1] -> 1 output column ~ 1 PE cycle),
    zero-initialized once by a zeros-stationary matmul.  The per-J
    reciprocal reads d_cols[:, J] directly -- no transpose bounce.
  - All matmul operands bf16 (1 cyc/row on the PE), fp32 accumulation;
    o_acc partials are bf16 (fits the pools; ~0.03% extra noise).
  - Queues: sync = wv first half/xkv pieces/masks -> key-chunk reads ->
    out stores; scalar = wM/xq loads -> V writebacks; gpsimd = wv
    second half -> collective triggers -> V gather reads.
  - Final PV pass runs J descending so the smallest accumulation group
    (and its store) lands last, trimming the drain tail.
"""
import sys

sys.path.insert(0, "/opt/trn_rl_repo")

import numpy as np
import ml_dtypes

import concourse.bacc as bacc
import concourse.mybir as mybir
import concourse.tile as tile
from concourse import bass_utils

S, D, NC = 8192, 1024, 8
QPC = S // NC  # 1024 queries (and kv rows) per core
NCH = D // 128  # 8 chunks of the feature dim
NQT = QPC // 128  # 8 query tiles per core
SCALE = 1.0 / np.sqrt(D).astype(np.float32)  # 1/32
BF16 = mybir.dt.bfloat16
F32 = mybir.dt.float32

# small diagonal shards interleaved with large ones so tiny
# matmul->exp->mask latency chains overlap big matmuls
H1_ORDER = [7, 0, 6, 1, 5, 2, 4, 3]

_cache = {}


def _j_groups(Sb):
    """Contiguous J-tile ranges covering J in [Sb, 8)."""
    if Sb + 4 < NQT:
        return [(Sb, Sb + 4), (Sb + 4, NQT)]
    return [(Sb, NQT)]


def _build():
    if "nc" in _cache:
        return _cache["nc"]
    nc = bacc.Bacc("TRN2", target_bir_lowering=False, debug=False, num_devices=NC)

    xt_full = nc.dram_tensor("xt_full", [D, S], BF16, kind="ExternalInput")
    xt_kv = nc.dram_tensor("xt_kv", [D, QPC], BF16, kind="ExternalInput")
    xt_q = nc.dram_tensor("xt_q", [D, QPC], BF16, kind="ExternalInput")
    wM = nc.dram_tensor("wM", [D, D], BF16, kind="ExternalInput")
    wvT = nc.dram_tensor("wvT", [D, D], BF16, kind="ExternalInput")
    masks = nc.dram_tensor("masks", [8, 128, 128], BF16, kind="ExternalInput")
    out = nc.dram_tensor("out", [QPC, D], F32, kind="ExternalOutput")

    rg = [list(range(NC))]

    def all_gather(src, dst):
        nc.gpsimd.collective_compute(
            "AllGather",
            mybir.AluOpType.bypass,
            replica_groups=rg,
            ins=[src.opt()],
            outs=[dst.opt()],
        )

    with tile.TileContext(nc) as tc:
        with tc.tile_pool(name="dram", bufs=1, space="DRAM") as dram:
            ag_va = dram.tile([256, D], BF16, name="agva")
            ag_vb = dram.tile([256, D], BF16, name="agvb")
            ag_v1 = dram.tile([512, D], BF16, name="agv1")
            g_va = dram.tile([NC, 256, D], BF16, addr_space="Shared", name="gva")
            g_vb = dram.tile([NC, 256, D], BF16, addr_space="Shared", name="gvb")
            g_v1 = dram.tile([NC, 512, D], BF16, addr_space="Shared", name="gv1")

            with (
                tc.tile_pool(name="persist", bufs=1) as persist,
                tc.tile_pool(name="fin", bufs=2) as fin,
                tc.tile_pool(name="ptp", bufs=1) as ptp,
                tc.tile_pool(name="kp", bufs=14) as kp,
            ):
                sb_qt = persist.tile([128, NCH * QPC], BF16, tag="qt")
                sb_mask = persist.tile([128, 8 * 128], BF16, tag="msk")
                sb_ones = persist.tile([128, 1], BF16, tag="ones")
                nc.vector.memset(sb_ones[:], 1.0)
                sb_zero128 = persist.tile([128, 128], BF16, tag="z128")
                nc.vector.memset(sb_zero128[:], 0.0)
                o_acc = [
                    persist.tile([128, D], BF16, tag=f"oacc{j}", name=f"oacc{j}")
                    for j in range(NQT)
                ]
                sb_rc = persist.tile([128, NQT], F32, tag="rc")

                def kt_read(col0):
                    """[128, 8, 256] key-chunk tile <- xt_full[:, col0:+256]."""
                    kt_t = kp.tile(
                        [128, NCH * 256], BF16, tag="kt", name=f"kt_{col0}"
                    )
                    nc.sync.dma_start(
                        kt_t.rearrange("p (c k) -> p c k", c=NCH),
                        xt_full[:, col0 : col0 + 256].rearrange(
                            "(c p) k -> p c k", p=128
                        ),
                    )
                    return kt_t

                # ---- projection phase ----
                with (
                    tc.tile_pool(name="io", bufs=1) as io,
                    tc.tile_pool(name="pp", bufs=6, space="PSUM") as pp,
                    tc.tile_pool(name="stage", bufs=4) as stage,
                ):
                    sb_xkv = io.tile([128, NCH * QPC], BF16, tag="xkv")
                    sb_xq = io.tile([128, NCH * QPC], BF16, tag="xq")
                    sb_wm = io.tile([128, NCH * D], BF16, tag="wm")
                    sb_wv = io.tile([128, NCH * D], BF16, tag="wv")
                    wv_c = sb_wv.rearrange("p (c k) -> p c k", c=NCH)
                    xkv_c = sb_xkv.rearrange("p (c k) -> p c k", c=NCH)
                    # Head loads spread across three queues so the first
                    # V-h0 group starts on ~0.75 MB:
                    #   sync:   wv[:,0:512] in two chunk pieces, wM, masks
                    #   scalar: xkv[:,0:512] in two pieces, xq
                    #   gpsimd: wv[:,512:], xkv[:,512:]
                    nc.sync.dma_start(
                        wv_c[:, 0:4, 0:512],
                        wvT[0:512, 0:512].rearrange("(c p) k -> p c k", p=128),
                    )
                    nc.scalar.dma_start(
                        xkv_c[:, :, 0:128],
                        xt_kv[:, 0:128].rearrange("(c p) k -> p c k", p=128),
                    )
                    nc.gpsimd.dma_start(
                        wv_c[:, :, 512:1024],
                        wvT[:, 512:1024].rearrange("(c p) k -> p c k", p=128),
                    )
                    nc.sync.dma_start(
                        wv_c[:, 4:8, 0:512],
                        wvT[512:1024, 0:512].rearrange("(c p) k -> p c k", p=128),
                    )
                    nc.scalar.dma_start(
                        xkv_c[:, :, 128:512],
                        xt_kv[:, 128:512].rearrange("(c p) k -> p c k", p=128),
                    )
                    nc.gpsimd.dma_start(
                        xkv_c[:, :, 512:1024],
                        xt_kv[:, 512:1024].rearrange("(c p) k -> p c k", p=128),
                    )
                    nc.sync.dma_start(
                        sb_wm.rearrange("p (c k) -> p c k", c=NCH),
                        wM.rearrange("(c p) k -> p c k", p=128),
                    )
                    nc.scalar.dma_start(
                        sb_xq.rearrange("p (c k) -> p c k", c=NCH),
                        xt_q.rearrange("(c p) k -> p c k", p=128),
                    )
                    nc.sync.dma_start(
                        sb_mask.rearrange("k (t q) -> k t q", t=8),
                        masks.rearrange("t k q -> k t q"),
                    )

                    # ST-h0 key-chunk reads: virgin SBUF (kp opened before
                    # io), queued on sync behind the head loads.
                    kt0_ts = {}
                    for piece in (0, 1):
                        for Sb in H1_ORDER:
                            kt0_ts[(piece, Sb)] = kt_read(Sb * QPC + piece * 256)

                    def proj_group1(lhs_sb, lhs_off, rhs_sb, rhs_off):
                        ps = pp.tile([128, 512], F32, tag="pp", name="ps")
                        for c in range(NCH):
                            nc.tensor.matmul(
                                ps[:],
                                lhs_sb[
                                    :, c * 1024 + lhs_off : c * 1024 + lhs_off + 128
                                ],
                                rhs_sb[
                                    :, c * 1024 + rhs_off : c * 1024 + rhs_off + 512
                                ],
                                start=(c == 0),
                                stop=(c == NCH - 1),
                            )
                        return ps

                    def proj_group2(lhs_sb, lhs_off, rhs_sb, rhs_off0, rhs_off1):
                        """Two 512-wide outputs sharing the stationary operand
                        (back-to-back matmuls reuse the loaded weights)."""
                        ps0 = pp.tile([128, 512], F32, tag="pp", name="ps0")
                        ps1 = pp.tile([128, 512], F32, tag="pp", name="ps1")
                        for c in range(NCH):
                            lhs = lhs_sb[
                                :, c * 1024 + lhs_off : c * 1024 + lhs_off + 128
                            ]
                            nc.tensor.matmul(
                                ps0[:],
                                lhs,
                                rhs_sb[
                                    :, c * 1024 + rhs_off0 : c * 1024 + rhs_off0 + 512
                                ],
                                start=(c == 0),
                                stop=(c == NCH - 1),
                            )
                            nc.tensor.matmul(
                                ps1[:],
                                lhs,
                                rhs_sb[
                                    :, c * 1024 + rhs_off1 : c * 1024 + rhs_off1 + 512
                                ],
                                start=(c == 0),
                                stop=(c == NCH - 1),
                            )
                        return ps0, ps1

                    def evict(ps, cols, dst_ap):
                        stg = stage.tile([128, cols], BF16, tag="stg", name="stg")
                        nc.vector.tensor_copy(stg[:], ps)
                        nc.scalar.dma_start(dst_ap, stg[:])

                    # V-h0: key chunks kc4; rows 0-1 -> piece a, 2-3 -> piece b.
                    # All dh=0 groups (wv first half) run before any dh=1
                    # group, so only wv[0:512] gates the projection start.
                    # The collectives can't begin before the CC engine's
                    # ~70us startup anyway, so the later va readiness is free.
                    for kc4 in range(4):
                        ps = proj_group1(sb_xkv, kc4 * 128, sb_wv, 0)
                        dst = ag_va if kc4 < 2 else ag_vb
                        evict(
                            ps[:],
                            512,
                            dst[(kc4 % 2) * 128 : (kc4 % 2) * 128 + 128, 0:512],
                        )
                    for kc4 in range(4):
                        ps = proj_group1(sb_xkv, kc4 * 128, sb_wv, 512)
                        dst = ag_va if kc4 < 2 else ag_vb
                        evict(
                            ps[:],
                            512,
                            dst[(kc4 % 2) * 128 : (kc4 % 2) * 128 + 128, 512:1024],
                        )
                        if kc4 == 1:
                            all_gather(ag_va, g_va)
                    all_gather(ag_vb, g_vb)

                    # V-h1
                    for kc4 in range(4):
                        ps0, ps1 = proj_group2(sb_xkv, 512 + kc4 * 128, sb_wv, 0, 512)
                        for dh, ps in ((0, ps0), (1, ps1)):
                            evict(
                                ps[:],
                                512,
                                ag_v1[
                                    kc4 * 128 : (kc4 + 1) * 128,
                                    dh * 512 : (dh + 1) * 512,
                                ],
                            )
                    all_gather(ag_v1, g_v1)

                    # Qt projection: Qt^T[b, q] = sum_a M[a, b] x_q^T[a, q];
                    # evicted straight into the persistent sb_qt layout.
                    for do in range(NCH):
                        ps0, ps1 = proj_group2(sb_wm, do * 128, sb_xq, 0, 512)
                        for h, ps in ((0, ps0), (1, ps1)):
                            nc.vector.tensor_copy(
                                sb_qt[
                                    :, do * QPC + h * 512 : do * QPC + h * 512 + 512
                                ],
                                ps[:],
                            )

                # ---- attention ----
                # vp inherits io's just-freed region: V gather reads WAR
                # only on the finished projections and fire immediately.
                vp_cm = tc.tile_pool(name="vp", bufs=16)
                vp = vp_cm.__enter__()
                v_ts = {}
                for piece, gv in ((0, g_va), (1, g_vb)):
                    for Sb in range(NC):
                        v_t = vp.tile(
                            [128, 2 * D], BF16, tag="v", name=f"v{piece}_{Sb}"
                        )
                        nc.gpsimd.dma_start(
                            v_t.rearrange("p (c d) -> p c d", c=2),
                            gv[Sb].rearrange("(c p) d -> p c d", p=128),
                        )
                        v_ts[(piece, Sb)] = v_t
                # V-h1 as two half-tiles per shard through the same pool
                # (slots WAR on PV-h0 readers and stream in behind them).
                for Sb in range(NC):
                    for half in range(2):
                        v_t = vp.tile(
                            [128, 2 * D], BF16, tag="v", name=f"v1_{Sb}_{half}"
                        )
                        nc.gpsimd.dma_start(
                            v_t.rearrange("p (c d) -> p c d", c=2),
                            g_v1[Sb][half * 256 : half * 256 + 256].rearrange(
                                "(c p) d -> p c d", p=128
                            ),
                        )
                        v_ts[(2 + half, Sb)] = v_t

                # ST-h1 key chunks through kp (two [*,256] tiles per shard).
                kt1_ts = {}
                for Sb in H1_ORDER:
                    for half in range(2):
                        kt1_ts[(Sb, half)] = kt_read(Sb * QPC + 512 + half * 256)

                # Denominator accumulator: one ones-column matmul per PV
                # chunk accumulates into column J.  Zero-initialized once
                # via a zeros-stationary matmul (PSUM start-zero granularity
                # is not per-column, so per-J start flags are unsafe).
                psdc_cm = tc.tile_pool(name="psdc", bufs=1, space="PSUM")
                psdc = psdc_cm.__enter__()
                d_cols = psdc.tile([128, NQT], F32, tag="dcols", name="d_cols")
                nc.tensor.matmul(
                    d_cols[:],
                    sb_zero128[:],
                    sb_mask[:, 0:NQT],
                    start=True,
                    stop=False,
                    skip_group_check=True,
                )

                pts = {}

                def st_groups(psst, Sb, kt, lhsT_of):
                    """Score-transpose groups for (shard Sb, key tile kt).

                    lhsT_of(c) gives the [128, 128] stationary Xk^T chunk.
                    Fills pts[(Sb, j0, kt)] with exp'd (masked) P tiles.
                    P tiles share tags across key-halves (same shapes), so
                    half-1 reuses half-0's buffers once PV-h0 is done.
                    """
                    for (j0, j1) in _j_groups(Sb):
                        N = (j1 - j0) * 128
                        # Diagonal group: queries below q0 are fully masked
                        # for this key tile (for every core: 128*kt > 8*q+7),
                        # so skip their ST columns; the pt prefix is zeroed
                        # explicitly (PSUM there is never written).
                        q0 = max(0, 16 * kt - 1) if j0 == Sb else 0
                        st = psst.tile([128, 512], F32, tag="st", name="st")
                        for c in range(NCH):
                            nc.tensor.matmul(
                                st[:, q0:N],
                                lhsT_of(c),
                                sb_qt[
                                    :,
                                    c * QPC + j0 * 128 + q0 : c * QPC + j1 * 128,
                                ],
                                start=(c == 0),
                                stop=(c == NCH - 1),
                            )
                        pt = ptp.tile(
                            [128, N],
                            BF16,
                            tag=f"pt{Sb}_{j0}_{kt % 4}",
                            name=f"pt{Sb}_{j0}_{kt}",
                        )
                        if q0 > 0:
                            nc.vector.memset(pt[:, 0:q0], 0.0)
                        nc.scalar.activation(
                            pt[:, q0:N],
                            st[:, q0:N],
                            mybir.ActivationFunctionType.Exp,
                            scale=float(SCALE),
                        )
                        if j0 == Sb:
                            nc.vector.tensor_mul(
                                pt[:, 0:128],
                                pt[:, 0:128],
                                sb_mask[:, kt * 128 : kt * 128 + 128],
                            )
                        pts[(Sb, j0, kt)] = pt

                def pv_pass(pso, v_of, kts, first, last):
                    """J-major PV: one PSUM group per J over all shards and
                    the key tiles `kts`; per chunk a 1-column ones matmul
                    accumulates the denominator into d_cols[:, J] while the
                    P tile is stationary.  Evict into o_acc (copy if
                    `first`), finalize + store if `last` (J descending so
                    the smallest group drains last)."""
                    j_order = range(NQT - 1, -1, -1) if last else range(NQT)
                    for J in j_order:
                        o_ps = pso.tile([128, 1024], F32, tag="ops", name="o_ps")
                        chunks = [(Sb, kt) for Sb in range(J + 1) for kt in kts]
                        for idx, (Sb, kt) in enumerate(chunks):
                            j0 = Sb if J < min(Sb + 4, NQT) else Sb + 4
                            pt = pts[(Sb, j0, kt)]
                            lhsT = pt[:, (J - j0) * 128 : (J - j0 + 1) * 128]
                            rhs = v_of(Sb, kt)
                            st_f = idx == 0
                            sp_f = idx == len(chunks) - 1
                            nc.tensor.matmul(
                                o_ps[:, 0:512], lhsT, rhs[0], start=st_f, stop=sp_f
                            )
                            nc.tensor.matmul(
                                o_ps[:, 512:1024], lhsT, rhs[1], start=st_f, stop=sp_f
                            )
                            nc.tensor.matmul(
                                d_cols[:, J : J + 1],
                                lhsT,
                                sb_ones[:],
                                start=False,
                                stop=last and sp_f,
                                skip_group_check=True,
                            )
                        if first:
                            nc.vector.tensor_copy(o_acc[J][:], o_ps[:])
                        else:
                            nc.vector.tensor_add(o_acc[J][:], o_acc[J][:], o_ps[:])
                        if last:
                            nc.vector.reciprocal(
                                sb_rc[:, J : J + 1], d_cols[:, J : J + 1]
                            )
                            outt = fin.tile([128, D], F32, tag="outt", name="outt")
                            nc.vector.tensor_scalar_mul(
                                outt[:], o_acc[J][:], sb_rc[:, J : J + 1]
                            )
                            nc.sync.dma_start(
                                out[J * 128 : (J + 1) * 128, :], outt[:]
                            )

                def v_slices(v_t, k2):
                    return (
                        v_t[:, k2 * D : k2 * D + 512],
                        v_t[:, k2 * D + 512 : k2 * D + 1024],
                    )

                # ---- ST half 0 (H1_ORDER interleaves tiny diagonal groups
                # between big ones so the exp activations keep pace) ----
                psst_cm = tc.tile_pool(name="psst0", bufs=6, space="PSUM")
                psst = psst_cm.__enter__()
                for piece in (0, 1):
                    for Sb in H1_ORDER:
                        kt_t = kt0_ts[(piece, Sb)]
                        for k2 in range(2):
                            kt = piece * 2 + k2
                            st_groups(
                                psst,
                                Sb,
                                kt,
                                lambda c, kt_t=kt_t, k2=k2: kt_t[
                                    :, c * 256 + k2 * 128 : c * 256 + k2 * 128 + 128
                                ],
                            )
                psst_cm.__exit__(None, None, None)

                # ---- PV half 0 ----
                pso_cm = tc.tile_pool(name="pso0", bufs=3, space="PSUM")
                pso = pso_cm.__enter__()

                def v_of0(piece):
                    def f(Sb, kt):
                        return v_slices(v_ts[(piece, Sb)], kt - piece * 2)

                    return f

                pv_pass(pso, v_of0(0), (0, 1), first=True, last=False)
                pv_pass(pso, v_of0(1), (2, 3), first=False, last=False)
                pso_cm.__exit__(None, None, None)

                # ---- ST half 1 ----
                psst_cm = tc.tile_pool(name="psst1", bufs=6, space="PSUM")
                psst = psst_cm.__enter__()
                for Sb in H1_ORDER:
                    for k4 in range(4):
                        kt = 4 + k4
                        kt_t = kt1_ts[(Sb, k4 // 2)]
                        st_groups(
                            psst,
                            Sb,
                            kt,
                            lambda c, kt_t=kt_t, k2=k4 % 2: kt_t[
                                :, c * 256 + k2 * 128 : c * 256 + k2 * 128 + 128
                            ],
                        )
                psst_cm.__exit__(None, None, None)

                # ---- PV half 1 + finalize ----
                pso_cm = tc.tile_pool(name="pso1", bufs=3, space="PSUM")
                pso = pso_cm.__enter__()

                def v_of1(Sb, kt):
                    return v_slices(v_ts[(2 + (kt - 4) // 2, Sb)], (kt - 4) % 2)

                pv_pass(pso, v_of1, (4, 5, 6, 7), first=False, last=True)
                pso_cm.__exit__(None, None, None)
                psdc_cm.__exit__(None, None, None)
                vp_cm.__exit__(None, None, None)

    nc.compile()
    _cache["nc"] = nc
    return nc


def _make_in_maps(inputs, w_query, w_key, w_value):
    bf = ml_dtypes.bfloat16
    xt = np.ascontiguousarray(inputs.T.astype(np.float32))  # [D, S]
    xt_b = np.ascontiguousarray(xt.astype(bf))
    # scores = (x_q M) x_k^T with M = Wq^T Wk
    wM = np.ascontiguousarray(
        w_query.T.astype(np.float32) @ w_key.astype(np.float32)
    ).astype(bf)
    wvT = np.ascontiguousarray(w_value.T).astype(bf)

    kt_off = np.arange(8)[:, None, None] * 128 + np.arange(128)[None, :, None]
    in_maps = []
    for i in range(NC):
        xkv = np.ascontiguousarray(xt_b[:, i * QPC : (i + 1) * QPC])
        xq = np.ascontiguousarray(xt_b[:, i::NC])
        q_off = np.arange(128)[None, None, :] * 8 + i
        m = (kt_off <= q_off).astype(np.float32).astype(bf)  # [8,128,128]
        in_maps.append(
            {
                "xt_full": xt_b,
                "xt_kv": xkv,
                "xt_q": xq,
                "wM": wM,
                "wvT": wvT,
                "masks": np.ascontiguousarray(m),
            }
        )
    return in_maps


def run(inputs, w_query, w_key, w_value, trace=False):
    nc = _build()
    in_maps = _make_in_maps(inputs, w_query, w_key, w_value)
    res = bass_utils.run_bass_kernel_spmd(
        nc, in_maps, core_ids=list(range(NC)), trace=trace
    )
    full = np.empty((S, D), dtype=np.float32)
    for i in range(NC):
        full[i::NC] = res.results[i]["out"]
    return full, res


def kernel(inputs, w_query, w_key, w_value):
    inputs = np.asarray(inputs, dtype=np.float32)
    w_query = np.asarray(w_query, dtype=np.float32)
    w_key = np.asarray(w_key, dtype=np.float32)
    w_value = np.asarray(w_value, dtype=np.float32)
    full, _ = run(inputs, w_query, w_key, w_value, trace=False)
    return full


# revision 4
# speedup vs baseline: 1.0009x; 1.0009x over previous
"""Causal self-attention (SEQ=8192, D=1024) on 8 TRN2 NeuronCores.

Strategy (SPMD, one static graph on all 8 cores), v7 "raw-keys" design:
  - Sequence parallel over queries with stride-8 row interleaving:
    core i owns query rows {8j+i : j in [0,1024)}. This balances causal
    work exactly while keeping the instruction graph identical across
    cores (per-core differences are pure data: X^T slices + masks).
  - The combined score weight is applied on the QUERY side:
    scores = (x_q M) x_k^T with M = Wq^T Wk, so the key-side ST operand
    is the RAW input X^T -- which every core receives as a (replicated)
    input. No K projection and no K all-gathers at all; ST reads key
    chunks straight from DRAM with no collective in the way.
  - Core i computes V for its contiguous key shard [1024*i, 1024*(i+1))
    only; V is shared via THREE chunked AllGathers (V-h0 in two 256-key
    pieces, V-h1). The CC engine takes ~70us to start its first
    collective regardless of trigger time, and the chain finishes by
    ~200us -- well before PV needs each piece.
  - ALL key-chunk tiles ([128, 8, 256] from X^T) stream through ONE
    pool (kp, 14 bufs) and ALL gathered-V tiles ([128, 2, 1024])
    through ONE pool (vp, 16 bufs): uniform tile sizes mean h1 data
    reuses h0 slots with natural WAR ordering, SBUF stays ~200 KB in
    every phase, and no read is throttled by pool-lifetime conflicts.
    kp opens before io (virgin SBUF: kt0 reads issue at t~10us); vp
    opens right after io closes and inherits its region (reads fire
    the moment the projections drain).
  - Projection order: V-h0 (dh=0 groups first: only wv's first half
    gates the start), V-h1, then Qt = x_q M evicted to SBUF.
  - Attention runs in S^T layout ([keys x queries]): S^T = Xk^T.T @ Qt^T,
    so softmax(P)^T is directly the lhsT for P@V -- no transposes.
    Per key-half: ST for all shards first (H1_ORDER interleaves tiny
    diagonal groups between big ones so the exp activations keep
    pace), then PV in J-major order with one PSUM accumulation group
    per (J, kt-piece) spanning all shards.  exp on ScalarE (scale
    fused), no max-subtraction (scores are N(0,1)-scaled).
  - Denominators: a [128, 8] PSUM tile accumulates one ones-COLUMN
    matmul per PV chunk (lhsT = the P tile already stationary for the
    PV matmuls, rhs = ones[128,1] -> 1 output column ~ 1 PE cycle),
    zero-initialized once by a zeros-stationary matmul.  The per-J
    reciprocal reads d_cols[:, J] directly -- no transpose bounce.
  - All matmul operands bf16 (1 cyc/row on the PE), fp32 accumulation;
    o_acc partials are bf16 (fits the pools; ~0.03% extra noise).
  - Queues: sync = wv first half/xkv pieces/masks -> key-chunk reads ->
    out stores; scalar = wM/xq loads -> V writebacks; gpsimd = wv
    second half -> collective triggers -> V gather reads.
  - Final PV pass runs J descending so the smallest accumulation group
    (and its store) lands last, trimming the drain tail.
"""
import sys

sys.path.insert(0, "/opt/trn_rl_repo")

import numpy as np
import ml_dtypes

import concourse.bacc as bacc
import concourse.mybir as mybir
import concourse.tile as tile
from concourse import bass_utils

S, D, NC = 8192, 1024, 8
QPC = S // NC  # 1024 queries (and kv rows) per core
NCH = D // 128  # 8 chunks of the feature dim
NQT = QPC // 128  # 8 query tiles per core
SCALE = 1.0 / np.sqrt(D).astype(np.float32)  # 1/32
BF16 = mybir.dt.bfloat16
F32 = mybir.dt.float32

# small diagonal shards interleaved with large ones so tiny
# matmul->exp->mask latency chains overlap big matmuls
H1_ORDER = [7, 0, 6, 1, 5, 2, 4, 3]

_cache = {}


def _j_groups(Sb):
    """Contiguous J-tile ranges covering J in [Sb, 8)."""
    if Sb + 4 < NQT:
        return [(Sb, Sb + 4), (Sb + 4, NQT)]
    return [(Sb, NQT)]


def _build():
    if "nc" in _cache:
        return _cache["nc"]
    nc = bacc.Bacc("TRN2", target_bir_lowering=False, debug=False, num_devices=NC)

    xt_full = nc.dram_tensor("xt_full", [D, S], BF16, kind="ExternalInput")
    xt_kv = nc.dram_tensor("xt_kv", [D, QPC], BF16, kind="ExternalInput")
    xt_q = nc.dram_tensor("xt_q", [D, QPC], BF16, kind="ExternalInput")
    wM = nc.dram_tensor("wM", [D, D], BF16, kind="ExternalInput")
    wvT = nc.dram_tensor("wvT", [D, D], BF16, kind="ExternalInput")
    masks = nc.dram_tensor("masks", [8, 128, 128], BF16, kind="ExternalInput")
    out = nc.dram_tensor("out", [QPC, D], F32, kind="ExternalOutput")

    rg = [list(range(NC))]

    def all_gather(src, dst):
        nc.gpsimd.collective_compute(
            "AllGather",
            mybir.AluOpType.bypass,
            replica_groups=rg,
            ins=[src.opt()],
            outs=[dst.opt()],
        )

    with tile.TileContext(nc) as tc:
        with tc.tile_pool(name="dram", bufs=1, space="DRAM") as dram:
            ag_va = dram.tile([256, D], BF16, name="agva")
            ag_vb = dram.tile([256, D], BF16, name="agvb")
            ag_v1 = dram.tile([512, D], BF16, name="agv1")
            g_va = dram.tile([NC, 256, D], BF16, addr_space="Shared", name="gva")
            g_vb = dram.tile([NC, 256, D], BF16, addr_space="Shared", name="gvb")
            g_v1 = dram.tile([NC, 512, D], BF16, addr_space="Shared", name="gv1")

            with (
                tc.tile_pool(name="persist", bufs=1) as persist,
                tc.tile_pool(name="fin", bufs=2) as fin,
                tc.tile_pool(name="ptp", bufs=1) as ptp,
                tc.tile_pool(name="kp", bufs=14) as kp,
            ):
                sb_qt = persist.tile([128, NCH * QPC], BF16, tag="qt")
                sb_mask = persist.tile([128, 8 * 128], BF16, tag="msk")
                sb_ones = persist.tile([128, 1], BF16, tag="ones")
                nc.vector.memset(sb_ones[:], 1.0)
                sb_zero128 = persist.tile([128, 128], BF16, tag="z128")
                nc.vector.memset(sb_zero128[:], 0.0)
                o_acc = [
                    persist.tile([128, D], BF16, tag=f"oacc{j}", name=f"oacc{j}")
                    for j in range(NQT)
                ]
                sb_rc = persist.tile([128, NQT], F32, tag="rc")

                def kt_read(col0):
                    """[128, 8, 256] key-chunk tile <- xt_full[:, col0:+256]."""
                    kt_t = kp.tile(
                        [128, NCH * 256], BF16, tag="kt", name=f"kt_{col0}"
                    )
                    nc.sync.dma_start(
                        kt_t.rearrange("p (c k) -> p c k", c=NCH),
                        xt_full[:, col0 : col0 + 256].rearrange(
                            "(c p) k -> p c k", p=128
                        ),
                    )
                    return kt_t

                # ---- projection phase ----
                with (
                    tc.tile_pool(name="io", bufs=1) as io,
                    tc.tile_pool(name="pp", bufs=6, space="PSUM") as pp,
                    tc.tile_pool(name="stage", bufs=4) as stage,
                ):
                    sb_xkv = io.tile([128, NCH * QPC], BF16, tag="xkv")
                    sb_xq = io.tile([128, NCH * QPC], BF16, tag="xq")
                    sb_wm = io.tile([128, NCH * D], BF16, tag="wm")
                    sb_wv = io.tile([128, NCH * D], BF16, tag="wv")
                    wv_c = sb_wv.rearrange("p (c k) -> p c k", c=NCH)
                    xkv_c = sb_xkv.rearrange("p (c k) -> p c k", c=NCH)
                    # Head loads spread across three queues so the first
                    # V-h0 group starts on ~0.75 MB:
                    #   sync:   wv[:,0:512] in two chunk pieces, wM, masks
                    #   scalar: xkv[:,0:512] in two pieces, xq
                    #   gpsimd: wv[:,512:], xkv[:,512:]
                    nc.sync.dma_start(
                        wv_c[:, 0:4, 0:512],
                        wvT[0:512, 0:512].rearrange("(c p) k -> p c k", p=128),
                    )
                    nc.scalar.dma_start(
                        xkv_c[:, :, 0:128],
                        xt_kv[:, 0:128].rearrange("(c p) k -> p c k", p=128),
                    )
                    nc.gpsimd.dma_start(
                        wv_c[:, :, 512:1024],
                        wvT[:, 512:1024].rearrange("(c p) k -> p c k", p=128),
                    )
                    nc.sync.dma_start(
                        wv_c[:, 4:8, 0:512],
                        wvT[512:1024, 0:512].rearrange("(c p) k -> p c k", p=128),
                    )
                    nc.scalar.dma_start(
                        xkv_c[:, :, 128:512],
                        xt_kv[:, 128:512].rearrange("(c p) k -> p c k", p=128),
                    )
                    nc.gpsimd.dma_start(
                        xkv_c[:, :, 512:1024],
                        xt_kv[:, 512:1024].rearrange("(c p) k -> p c k", p=128),
                    )
                    nc.sync.dma_start(
                        sb_wm.rearrange("p (c k) -> p c k", c=NCH),
                        wM.rearrange("(c p) k -> p c k", p=128),
                    )
                    nc.scalar.dma_start(
                        sb_xq.rearrange("p (c k) -> p c k", c=NCH),
                        xt_q.rearrange("(c p) k -> p c k", p=128),
                    )
                    nc.sync.dma_start(
                        sb_mask.rearrange("k (t q) -> k t q", t=8),
                        masks.rearrange("t k q -> k t q"),
                    )

                    # ST-h0 key-chunk reads: virgin SBUF (kp opened before
                    # io), queued on sync behind the head loads.
                    kt0_ts = {}
                    for piece in (0, 1):
                        for Sb in H1_ORDER:
                            kt0_ts[(piece, Sb)] = kt_read(Sb * QPC + piece * 256)

                    def proj_group1(lhs_sb, lhs_off, rhs_sb, rhs_off):
                        ps = pp.tile([128, 512], F32, tag="pp", name="ps")
                        for c in range(NCH):
                            nc.tensor.matmul(
                                ps[:],
                                lhs_sb[
                                    :, c * 1024 + lhs_off : c * 1024 + lhs_off + 128
                                ],
                                rhs_sb[
                                    :, c * 1024 + rhs_off : c * 1024 + rhs_off + 512
                                ],
                                start=(c == 0),
                                stop=(c == NCH - 1),
                            )
                        return ps

                    def proj_group2(lhs_sb, lhs_off, rhs_sb, rhs_off0, rhs_off1):
                        """Two 512-wide outputs sharing the stationary operand
                        (back-to-back matmuls reuse the loaded weights)."""
                        ps0 = pp.tile([128, 512], F32, tag="pp", name="ps0")
                        ps1 = pp.tile([128, 512], F32, tag="pp", name="ps1")
                        for c in range(NCH):
                            lhs = lhs_sb[
                                :, c * 1024 + lhs_off : c * 1024 + lhs_off + 128
                            ]
                            nc.tensor.matmul(
                                ps0[:],
                                lhs,
                                rhs_sb[
                                    :, c * 1024 + rhs_off0 : c * 1024 + rhs_off0 + 512
                                ],
                                start=(c == 0),
                                stop=(c == NCH - 1),
                            )
                            nc.tensor.matmul(
                                ps1[:],
                                lhs,
                                rhs_sb[
                                    :, c * 1024 + rhs_off1 : c * 1024 + rhs_off1 + 512
                                ],
                                start=(c == 0),
                                stop=(c == NCH - 1),
                            )
                        return ps0, ps1

                    def evict(ps, cols, dst_ap):
                        stg = stage.tile([128, cols], BF16, tag="stg", name="stg")
                        nc.vector.tensor_copy(stg[:], ps)
                        nc.scalar.dma_start(dst_ap, stg[:])

                    # V-h0: key chunks kc4; rows 0-1 -> piece a, 2-3 -> piece b.
                    # All dh=0 groups (wv first half) run before any dh=1
                    # group, so only wv[0:512] gates the projection start.
                    # The collectives can't begin before the CC engine's
                    # ~70us startup anyway, so the later va readiness is free.
                    for kc4 in range(4):
                        ps = proj_group1(sb_xkv, kc4 * 128, sb_wv, 0)
                        dst = ag_va if kc4 < 2 else ag_vb
                        evict(
                            ps[:],
                            512,
                            dst[(kc4 % 2) * 128 : (kc4 % 2) * 128 + 128, 0:512],
                        )
                    for kc4 in range(4):
                        ps = proj_group1(sb_xkv, kc4 * 128, sb_wv, 512)
                        dst = ag_va if kc4 < 2 else ag_vb
                        evict(
                            ps[:],
                            512,
                            dst[(kc4 % 2) * 128 : (kc4 % 2) * 128 + 128, 512:1024],
                        )
                        if kc4 == 1:
                            all_gather(ag_va, g_va)
                    all_gather(ag_vb, g_vb)

                    # V-h1
                    for kc4 in range(4):
                        ps0, ps1 = proj_group2(sb_xkv, 512 + kc4 * 128, sb_wv, 0, 512)
                        for dh, ps in ((0, ps0), (1, ps1)):
                            evict(
                                ps[:],
                                512,
                                ag_v1[
                                    kc4 * 128 : (kc4 + 1) * 128,
                                    dh * 512 : (dh + 1) * 512,
                                ],
                            )
                    all_gather(ag_v1, g_v1)

                    # Qt projection: Qt^T[b, q] = sum_a M[a, b] x_q^T[a, q];
                    # evicted straight into the persistent sb_qt layout.
                    for do in range(NCH):
                        ps0, ps1 = proj_group2(sb_wm, do * 128, sb_xq, 0, 512)
                        for h, ps in ((0, ps0), (1, ps1)):
                            nc.vector.tensor_copy(
                                sb_qt[
                                    :, do * QPC + h * 512 : do * QPC + h * 512 + 512
                                ],
                                ps[:],
                            )

                # ---- attention ----
                # vp inherits io's just-freed region: V gather reads WAR
                # only on the finished projections and fire immediately.
                vp_cm = tc.tile_pool(name="vp", bufs=16)
                vp = vp_cm.__enter__()
                v_ts = {}
                for piece, gv in ((0, g_va), (1, g_vb)):
                    for Sb in range(NC):
                        v_t = vp.tile(
                            [128, 2 * D], BF16, tag="v", name=f"v{piece}_{Sb}"
                        )
                        nc.gpsimd.dma_start(
                            v_t.rearrange("p (c d) -> p c d", c=2),
                            gv[Sb].rearrange("(c p) d -> p c d", p=128),
                        )
                        v_ts[(piece, Sb)] = v_t
                # V-h1 as two half-tiles per shard through the same pool
                # (slots WAR on PV-h0 readers and stream in behind them).
                for Sb in range(NC):
                    for half in range(2):
                        v_t = vp.tile(
                            [128, 2 * D], BF16, tag="v", name=f"v1_{Sb}_{half}"
                        )
                        nc.gpsimd.dma_start(
                            v_t.rearrange("p (c d) -> p c d", c=2),
                            g_v1[Sb][half * 256 : half * 256 + 256].rearrange(
                                "(c p) d -> p c d", p=128
                            ),
                        )
                        v_ts[(2 + half, Sb)] = v_t

                # ST-h1 key chunks through kp (two [*,256] tiles per shard).
                kt1_ts = {}
                for Sb in H1_ORDER:
                    for half in range(2):
                        kt1_ts[(Sb, half)] = kt_read(Sb * QPC + 512 + half * 256)

                # Denominator accumulator: one ones-column matmul per PV
                # chunk accumulates into column J.  Zero-initialized once
                # via a zeros-stationary matmul (PSUM start-zero granularity
                # is not per-column, so per-J start flags are unsafe).
                psdc_cm = tc.tile_pool(name="psdc", bufs=1, space="PSUM")
                psdc = psdc_cm.__enter__()
                d_cols = psdc.tile([128, NQT], F32, tag="dcols", name="d_cols")
                nc.tensor.matmul(
                    d_cols[:],
                    sb_zero128[:],
                    sb_mask[:, 0:NQT],
                    start=True,
                    stop=False,
                    skip_group_check=True,
                )

                pts = {}

                def st_groups(psst, Sb, kt, lhsT_of):
                    """Score-transpose groups for (shard Sb, key tile kt).

                    lhsT_of(c) gives the [128, 128] stationary Xk^T chunk.
                    Fills pts[(Sb, j0, kt)] with exp'd (masked) P tiles.
                    P tiles share tags across key-halves (same shapes), so
                    half-1 reuses half-0's buffers once PV-h0 is done.
                    """
                    for (j0, j1) in _j_groups(Sb):
                        N = (j1 - j0) * 128
                        # Diagonal group: queries below q0 are fully masked
                        # for this key tile (for every core: 128*kt > 8*q+7),
                        # so skip their ST columns; the pt prefix is zeroed
                        # explicitly (PSUM there is never written).
                        q0 = max(0, 16 * kt - 1) if j0 == Sb else 0
                        st = psst.tile([128, 512], F32, tag="st", name="st")
                        for c in range(NCH):
                            nc.tensor.matmul(
                                st[:, q0:N],
                                lhsT_of(c),
                                sb_qt[
                                    :,
                                    c * QPC + j0 * 128 + q0 : c * QPC + j1 * 128,
                                ],
                                start=(c == 0),
                                stop=(c == NCH - 1),
                            )
                        pt = ptp.tile(
                            [128, N],
                            BF16,
                            tag=f"pt{Sb}_{j0}_{kt % 4}",
                            name=f"pt{Sb}_{j0}_{kt}",
                        )
                        if q0 > 0:
                            nc.vector.memset(pt[:, 0:q0], 0.0)
                        nc.scalar.activation(
                            pt[:, q0:N],
                            st[:, q0:N],
                            mybir.ActivationFunctionType.Exp,
                            scale=float(SCALE),
                        )
                        if j0 == Sb:
                            nc.vector.tensor_mul(
                                pt[:, 0:128],
                                pt[:, 0:128],
                                sb_mask[:, kt * 128 : kt * 128 + 128],
                            )
                        pts[(Sb, j0, kt)] = pt

                def pv_pass(pso, v_of, kts, first, last):
                    """J-major PV: one PSUM group per J over all shards and
                    the key tiles `kts`; per chunk a 1-column ones matmul
                    accumulates the denominator into d_cols[:, J] while the
                    P tile is stationary.  Evict into o_acc (copy if
                    `first`), finalize + store if `last` (J descending so
                    the smallest group drains last)."""
                    j_order = range(NQT - 1, -1, -1) if last else range(NQT)
                    for J in j_order:
                        o_ps = pso.tile([128, 1024], F32, tag="ops", name="o_ps")
                        chunks = [(Sb, kt) for Sb in range(J + 1) for kt in kts]
                        for idx, (Sb, kt) in enumerate(chunks):
                            j0 = Sb if J < min(Sb + 4, NQT) else Sb + 4
                            pt = pts[(Sb, j0, kt)]
                            lhsT = pt[:, (J - j0) * 128 : (J - j0 + 1) * 128]
                            rhs = v_of(Sb, kt)
                            st_f = idx == 0
                            sp_f = idx == len(chunks) - 1
                            nc.tensor.matmul(
                                o_ps[:, 0:512], lhsT, rhs[0], start=st_f, stop=sp_f
                            )
                            nc.tensor.matmul(
                                o_ps[:, 512:1024], lhsT, rhs[1], start=st_f, stop=sp_f
                            )
                            nc.tensor.matmul(
                                d_cols[:, J : J + 1],
                                lhsT,
                                sb_ones[:],
                                start=False,
                                stop=last and sp_f,
                                skip_group_check=True,
                            )
                        if first:
                            nc.vector.tensor_copy(o_acc[J][:], o_ps[:])
                        else:
                            nc.vector.tensor_add(o_acc[J][:], o_acc[J][:], o_ps[:])
                        if last:
                            nc.vector.reciprocal(
                                sb_rc[:, J : J + 1], d_cols[:, J : J + 1]
                            )
                            outt = fin.tile([128, D], F32, tag="outt", name="outt")
                            nc.vector.tensor_scalar_mul(
                                outt[:], o_acc[J][:], sb_rc[:, J : J + 1]
                            )
                            nc.sync.dma_start(
                                out[J * 128 : (J + 1) * 128, :], outt[:]
                            )

                def v_slices(v_t, k2):
                    return (
                        v_t[:, k2 * D : k2 * D + 512],
                        v_t[:, k2 * D + 512 : k2 * D + 1024],
                    )

                # ---- ST half 0 (H1_ORDER interleaves tiny diagonal groups
                # between big ones so the exp activations keep pace) ----
                psst_cm = tc.tile_pool(name="psst0", bufs=6, space="PSUM")
                psst = psst_cm.__enter__()
                for piece in (0, 1):
                    for Sb in H1_ORDER:
                        kt_t = kt0_ts[(piece, Sb)]
                        for k2 in range(2):
                            kt = piece * 2 + k2
                            st_groups(
                                psst,
                                Sb,
                                kt,
                                lambda c, kt_t=kt_t, k2=k2: kt_t[
                                    :, c * 256 + k2 * 128 : c * 256 + k2 * 128 + 128
                                ],
                            )
                psst_cm.__exit__(None, None, None)

                # ---- PV half 0 ----
                pso_cm = tc.tile_pool(name="pso0", bufs=3, space="PSUM")
                pso = pso_cm.__enter__()

                def v_of0(piece):
                    def f(Sb, kt):
                        return v_slices(v_ts[(piece, Sb)], kt - piece * 2)

                    return f

                pv_pass(pso, v_of0(0), (0, 1), first=True, last=False)
                pv_pass(pso, v_of0(1), (2, 3), first=False, last=False)
                pso_cm.__exit__(None, None, None)

                # ---- ST half 1 ----
                psst_cm = tc.tile_pool(name="psst1", bufs=6, space="PSUM")
                psst = psst_cm.__enter__()
                for Sb in H1_ORDER:
                    for k4 in range(4):
                        kt = 4 + k4
                        kt_t = kt1_ts[(Sb, k4 // 2)]
                        st_groups(
                            psst,
                            Sb,
                            kt,
                            lambda c, kt_t=kt_t, k2=k4 % 2: kt_t[
                                :, c * 256 + k2 * 128 : c * 256 + k2 * 128 + 128
                            ],
                        )
                psst_cm.__exit__(None, None, None)

                # ---- PV half 1 + finalize ----
                pso_cm = tc.tile_pool(name="pso1", bufs=3, space="PSUM")
                pso = pso_cm.__enter__()

                def v_of1(Sb, kt):
                    return v_slices(v_ts[(2 + (kt - 4) // 2, Sb)], (kt - 4) % 2)

                pv_pass(pso, v_of1, (4, 5, 6, 7), first=False, last=True)
                pso_cm.__exit__(None, None, None)
                psdc_cm.__exit__(None, None, None)
                vp_cm.__exit__(None, None, None)

    nc.compile()
    _cache["nc"] = nc
    return nc


def _make_in_maps(inputs, w_query, w_key, w_value):
    bf = ml_dtypes.bfloat16
    xt = np.ascontiguousarray(inputs.T.astype(np.float32))  # [D, S]
    xt_b = np.ascontiguousarray(xt.astype(bf))
    # scores = (x_q M) x_k^T with M = Wq^T Wk
    wM = np.ascontiguousarray(
        w_query.T.astype(np.float32) @ w_key.astype(np.float32)
    ).astype(bf)
    wvT = np.ascontiguousarray(w_value.T).astype(bf)

    kt_off = np.arange(8)[:, None, None] * 128 + np.arange(128)[None, :, None]
    in_maps = []
    for i in range(NC):
        xkv = np.ascontiguousarray(xt_b[:, i * QPC : (i + 1) * QPC])
        xq = np.ascontiguousarray(xt_b[:, i::NC])
        q_off = np.arange(128)[None, None, :] * 8 + i
        m = (kt_off <= q_off).astype(np.float32).astype(bf)  # [8,128,128]
        in_maps.append(
            {
                "xt_full": xt_b,
                "xt_kv": xkv,
                "xt_q": xq,
                "wM": wM,
                "wvT": wvT,
                "masks": np.ascontiguousarray(m),
            }
        )
    return in_maps


def run(inputs, w_query, w_key, w_value, trace=False):
    nc = _build()
    in_maps = _make_in_maps(inputs, w_query, w_key, w_value)
    res = bass_utils.run_bass_kernel_spmd(
        nc, in_maps, core_ids=list(range(NC)), trace=trace
    )
    full = np.empty((S, D), dtype=np.float32)
    for i in range(NC):
        full[i::NC] = res.results[i]["out"]
    return full, res


def kernel(inputs, w_query, w_key, w_value):
    inputs = np.asarray(inputs, dtype=np.float32)
    w_query = np.asarray(w_query, dtype=np.float32)
    w_key = np.asarray(w_key, dtype=np.float32)
    w_value = np.asarray(w_value, dtype=np.float32)
    full, _ = run(inputs, w_query, w_key, w_value, trace=False)
    return full
